# revision 11
# baseline (speedup 1.0000x reference)
"""Trainium2 Bass kernel for nn_BoardEncoder (HexConv board encoder).

Math:
  h[b,n,:] = relu(x[b,n] @ Wc.T + sum_k neighbors[b,n,k] @ Wd[k].T + bc + bd.sum(0))
  out[b]   = h[b].reshape(216) @ Wf.T + bf

Strategy (pure data-parallel over batch, 8 cores x 2048 rows):
  - Host packs per-(b,n) token features [x | neighbors | 1.0] (449 feats)
    into bf16, feature-major, chunked 128+128+128+65 so every load DMA
    covers full partitions:
      xtA[n, p, c*BS + b] = feat[c*128 + p]   (c in 0..2)   [N,128,3*BS]
      xtB[n, p, b]        = feat[384 + p]                    [N, 65,  BS]
    bf16 halves HBM traffic (the roofline for this memory-bound problem)
    and runs the PE at 1 cycle/row instead of fp32's 4.
  - Loads ride the sync (SP) HWDGE ring: one InstDMACopy is split across
    all 16 SDMA engines (the old 29-partition gpsimd slicing pinned 3
    engines at ~24 GB/s = 72 GB/s aggregate). Scatters + output stores
    ride the scalar (ACT) HWDGE ring so their sem-waits can't
    head-of-line-block the load FIFO.
  - Stage 1 (per cell n): psum[4,512] accumulates 4 chunk matmuls,
    relu -> bf16 strip [4,2048], SBUF->SBUF scatter to partition 4n of
    the h^T [(n,h), b] accumulator (hA rows 0..127, hB rows 128..215,
    hB row 88 stays 1.0 to provide the bf bias in stage 2).
  - Stage 2: out[128b,256] = hA_t.T @ wfta + hB_t.T @ wftb (bf16, f32
    psum), copied to f32 and stored.
"""

import sys

sys.path.insert(0, "/opt/trn_rl_repo")

import numpy as np

B = 16384
N = 54
D_IN = 64
KN = 6
D_HID = 4
D_OUT = 256
NCORES = 8
BS = B // NCORES          # 2048 batch rows per core
F = D_IN + KN * D_IN + 1  # 449 features incl. constant-1 bias feature
CA = 128                  # partition size of chunks 0..2
NCA = 3
CB = F - NCA * CA         # 65 = chunk-3 partition size
BT = 512                  # stage-1 moving free dim (tokens per matmul)
NBT = BS // BT            # 4
PAIR = 2                  # cells loaded per DMA (bigger descriptors)
NPAIR = N // PAIR         # 27

LAST_EXEC_NS = None

_PROGRAM = None


def _build_program(load_eng="sync"):
    """load_eng: which engine issues the big xt load DMAs
    ("sync" = SP HWDGE ring, "gpsimd" = SWDGE)."""
    import concourse.bacc as bacc
    import concourse.tile as tile
    from concourse import mybir

    f32 = mybir.dt.float32
    bf16 = mybir.dt.bfloat16

    nc = bacc.Bacc("TRN2", target_bir_lowering=False, debug=False,
                   num_devices=NCORES)
    xta_d = nc.declare_dram_parameter("xta", [NPAIR, CA, PAIR * NCA * BS],
                                      bf16, isOutput=False)
    xtb_d = nc.declare_dram_parameter("xtb", [NPAIR, CB, PAIR * BS], bf16,
                                      isOutput=False)
    wa_d = nc.declare_dram_parameter("wa", [CA, NCA * D_HID], bf16,
                                     isOutput=False)
    wb_d = nc.declare_dram_parameter("wb", [CB, D_HID], bf16,
                                     isOutput=False)
    wfta_d = nc.declare_dram_parameter("wfta", [128, D_OUT], bf16,
                                       isOutput=False)
    wftb_d = nc.declare_dram_parameter("wftb", [89, D_OUT], bf16,
                                       isOutput=False)
    out_d = nc.declare_dram_parameter("out", [BS, D_OUT], f32, isOutput=True)

    with tile.TileContext(nc) as tc:
        with (
            tc.tile_pool(name="consts", bufs=1) as consts,
            tc.tile_pool(name="hacc", bufs=1) as hacc,
            tc.tile_pool(name="xta", bufs=3) as xtap,
            tc.tile_pool(name="xtb", bufs=3) as xtbp,
            tc.tile_pool(name="hn", bufs=4) as hnp,
            tc.tile_pool(name="ps1", bufs=4, space="PSUM") as ps1,
            tc.tile_pool(name="ps2", bufs=2, space="PSUM") as ps2,
            tc.tile_pool(name="outp", bufs=3) as outp,
        ):
            wa_sb = consts.tile([CA, NCA * D_HID], bf16, tag="wa")
            nc.scalar.dma_start(wa_sb[:], wa_d[:])
            wb_sb = consts.tile([CB, D_HID], bf16, tag="wb")
            nc.scalar.dma_start(wb_sb[:], wb_d[:])
            wfta_sb = consts.tile([128, D_OUT], bf16, tag="wfta")
            nc.scalar.dma_start(wfta_sb[:], wfta_d[:])
            wftb_sb = consts.tile([89, D_OUT], bf16, tag="wftb")
            nc.scalar.dma_start(wftb_sb[:], wftb_d[:])

            hA = hacc.tile([128, BS], bf16, tag="hA")  # (n,h) rows 0..127
            hB = hacc.tile([89, BS], bf16, tag="hB")   # rows 128..215+ones
            # rows 0..87 are overwritten by the per-cell scatter DMAs
            # below; row 88 keeps the 1.0 fill and provides the bf bias
            # in stage 2. (a [88:89] memset is rejected: compute-engine
            # partition bases must be 32-aligned)
            nc.vector.memset(hB[:, :], 1.0)

            ld = nc.sync if load_eng == "sync" else nc.gpsimd

            for g in range(NPAIR):
                xta = xtap.tile([CA, PAIR * NCA * BS], bf16)
                ld.dma_start(xta[:], xta_d[g])
                xtb = xtbp.tile([CB, PAIR * BS], bf16)
                ld.dma_start(xtb[:], xtb_d[g])
                for j in range(PAIR):
                    n = PAIR * g + j
                    hn = hnp.tile([D_HID, BS], bf16)
                    for bt in range(NBT):
                        ps = ps1.tile([D_HID, BT], f32)
                        for c in range(NCA):
                            o = (j * NCA + c) * BS + bt * BT
                            nc.tensor.matmul(
                                ps[:],
                                wa_sb[:, c * D_HID:(c + 1) * D_HID],
                                xta[:, o:o + BT],
                                start=(c == 0),
                                stop=False,
                            )
                        nc.tensor.matmul(
                            ps[:],
                            wb_sb[:],
                            xtb[:, j * BS + bt * BT:j * BS + (bt + 1) * BT],
                            start=False,
                            stop=True,
                        )
                        dst = hn[:, bt * BT:(bt + 1) * BT]
                        if n % 2 == 0:
                            nc.vector.tensor_scalar_max(dst, ps[:], 0.0)
                        else:
                            nc.scalar.activation(
                                dst, ps[:],
                                mybir.ActivationFunctionType.Relu)
                    # scatter on scalar/ACT HWDGE ring: its sem-wait on
                    # the relu must not block the big-input-load FIFO
                    # (loads live on the sync/SP ring)
                    if n < 32:
                        nc.scalar.dma_start(hA[n * 4:(n + 1) * 4, :], hn[:])
                    else:
                        m = n - 32
                        nc.scalar.dma_start(hB[m * 4:(m + 1) * 4, :], hn[:])

            for t in range(BS // 128):
                po = ps2.tile([128, D_OUT], f32)
                nc.tensor.matmul(po[:], hA[:, t * 128:(t + 1) * 128],
                                 wfta_sb[:], start=True, stop=False)
                nc.tensor.matmul(po[:], hB[:, t * 128:(t + 1) * 128],
                                 wftb_sb[:], start=False, stop=True)
                ot = outp.tile([128, D_OUT], f32)
                if t % 2 == 0:
                    nc.vector.tensor_copy(ot[:], po[:])
                else:
                    nc.scalar.copy(ot[:], po[:])
                nc.scalar.dma_start(out_d[t * 128:(t + 1) * 128, :], ot[:])

    nc.compile()
    return nc


def _get_program():
    global _PROGRAM
    if _PROGRAM is None:
        _PROGRAM = _build_program()
    return _PROGRAM


def _pack_inputs(x, neighbors):
    """Per-shard bf16 feature-major packing:
    xtA[n, p, c*BS + b] = feat[c*128 + p] (c<3), xtB[n, p, b] = feat[384+p]
    of batch row (shard*BS + b), cell n. feat = [x | neighbors | 1]."""
    import ml_dtypes

    bf16 = ml_dtypes.bfloat16
    xbf = x.astype(bf16)                                    # [B, N, 64]
    nbf = neighbors.reshape(B, N, KN * D_IN).astype(bf16)   # [B, N, 384]
    packs = []
    for s in range(NCORES):
        sl = slice(s * BS, (s + 1) * BS)
        feats = np.concatenate(
            [xbf[sl], nbf[sl], np.ones((BS, N, 1), bf16)], axis=2
        )  # [BS, N, 449]
        # chunks 0..2, cells paired: -> [27, 128, (j,c,b)] = [27,128,2*3*BS]
        xta = np.ascontiguousarray(
            feats[:, :, :NCA * CA].reshape(BS, NPAIR, PAIR, NCA, CA)
            .transpose(1, 4, 2, 3, 0)).reshape(NPAIR, CA, PAIR * NCA * BS)
        # chunk 3, cells paired: -> [27, 65, (j,b)] = [27,65,2*BS]
        xtb = np.ascontiguousarray(
            feats[:, :, NCA * CA:].reshape(BS, NPAIR, PAIR, CB)
            .transpose(1, 3, 2, 0)).reshape(NPAIR, CB, PAIR * BS)
        packs.append((xta, xtb))
    return packs


def _pack_weights(Wc, bc, Wd, bd, Wf, bf):
    import ml_dtypes

    bf16 = ml_dtypes.bfloat16
    W_all = np.zeros((F, D_HID), np.float32)
    W_all[:D_IN] = Wc.T
    W_all[D_IN:F - 1] = Wd.transpose(0, 2, 1).reshape(KN * D_IN, D_HID)
    W_all[F - 1] = bc + bd.sum(0)
    # wa[p, c*4+h] = W_all[c*128+p, h]; wb[p, h] = W_all[384+p, h]
    wa = np.ascontiguousarray(
        W_all[:NCA * CA].reshape(NCA, CA, D_HID).transpose(1, 0, 2)
    ).reshape(CA, NCA * D_HID).astype(bf16)
    wb = W_all[NCA * CA:].astype(bf16)
    WfT = np.ascontiguousarray(Wf.T)            # [216, 256]
    wfta = WfT[:128].astype(bf16)
    wftb = np.concatenate([WfT[128:], bf[None, :]], axis=0)  # [89, 256]
    wftb = wftb.astype(bf16)
    return wa, wb, wfta, wftb


def kernel(x, neighbors, Wc, bc, Wd, bd, Wf, bf):
    global LAST_EXEC_NS
    from concourse.bass_utils import run_bass_kernel_spmd

    x = np.asarray(x, np.float32)
    neighbors = np.asarray(neighbors, np.float32)
    wa, wb, wfta, wftb = _pack_weights(
        np.asarray(Wc, np.float32), np.asarray(bc, np.float32),
        np.asarray(Wd, np.float32), np.asarray(bd, np.float32),
        np.asarray(Wf, np.float32), np.asarray(bf, np.float32))
    packs = _pack_inputs(x, neighbors)

    nc = _get_program()
    in_maps = [
        {"xta": packs[s][0], "xtb": packs[s][1], "wa": wa, "wb": wb,
         "wfta": wfta, "wftb": wftb}
        for s in range(NCORES)
    ]
    res = run_bass_kernel_spmd(nc, in_maps, list(range(NCORES)))
    LAST_EXEC_NS = res.exec_time_ns
    out = np.concatenate([res.results[s]["out"] for s in range(NCORES)],
                         axis=0)
    return out
